# revision 21
# baseline (speedup 1.0000x reference)
"""LISTA (learned ISTA) sparse-coding forward pass on 8 Trainium2 NeuronCores.

Problem: I [4,1,192,192] -> im2col(9x9) -> 24 soft-thresholded iterations over
64 filters -> decode -> col2im overlap-add average -> [4,1,192,192].

Sharding: 8 cores = 4 images x 2 position-row halves (92 rows of 184 positions
each). Each core computes its full LISTA pipeline plus the col2im partial sums
for its 100-row output slab; the host merges the 8-row seams between the two
slabs of each image and divides by the overlap counts (pure unshard glue).

Algebra used (exact rewrites of the reference up to fp assoc.):
  - mean-subtraction folded into encoder:  c = WAc @ I_col,
      WAc = WA - rowmean(WA)  (since mean_patch = (1/81) * ones^T I_col)
  - iteration fused:  gamma_{t+1} = soft(S @ gamma_t + c),  S = I - WA@WD
  - gamma kept as a + bneg with a = relu(y-l) >= 0, bneg = min(y+l, 0) <= 0
    (soft(y) = a + bneg), so the subtraction never needs its own pass:
      y_{t+1} = Id@c + S@a_t + S@bneg_t   (3 accumulating PE passes)
  - decode: out_all = WW@a + WW@bneg + (J/81) @ I_col   (mean add-back)
"""

import contextlib
import numpy as np

# ---------------------------------------------------------------- constants
B, H, Wimg = 4, 192, 192
K = 9
F = 64
NCH = K * K  # 81
HO = H - K + 1  # 184
WO = Wimg - K + 1  # 184
UNF = 24
N_CORES = 8

ROWS = HO // 2  # 92 position rows per core
SLAB = ROWS + K - 1  # 100 image/output rows per core
NPOS = ROWS * WO  # 16928 positions per core
HALFR = ROWS // 2  # 46 rows per block-diag half
HALF = HALFR * WO  # 8464 columns per half

CH = 512
CHUNKS = [(i * CH, min((i + 1) * CH, HALF)) for i in range((HALF + CH - 1) // CH)]
SUPERS = [CHUNKS[i : i + 4] for i in range(0, len(CHUNKS), 4)]
DROWS = 2  # decode chunk = 2 position rows
DCH = DROWS * WO  # 368 columns

# weight blob layout: (name, partitions, cols) — bf16
BLOB_SPEC = [
    ("wac", NCH, F), ("wacp", NCH, 128), ("sbd", 128, 128), ("id128", 128, 128),
    ("wwa", 128, NCH), ("wwb", 128, NCH), ("j81", NCH, NCH),
    ("eshb", ROWS, K * SLAB),
]
BLOBC = sum(nf for _, _, nf in BLOB_SPEC)

_STATE = {}


def _split_multi_waits(nc, mybir):
    """This walrus build supports a single sync-wait slot per instruction.
    Move extra waits onto preceding same-engine no-ops (same semantics:
    program order on one engine; all waits clear before the instruction)."""
    cnt = 0
    for fn in nc.m.functions:
        for bb in fn.blocks:
            insts = bb.instructions
            need = False
            for ins in insts:
                si = ins.sync_info
                if si is not None and si.on_wait is not None and len(si.on_wait) > 1:
                    need = True
                    break
            if not need:
                continue
            out = []
            for ins in insts:
                si = ins.sync_info
                if si is not None and si.on_wait is not None and len(si.on_wait) > 1:
                    waits = list(si.on_wait)
                    for w in waits[:-1]:
                        cnt += 1
                        nop = mybir.InstNoOp(name=f"wsplit-{cnt}", ins=[], outs=[])
                        nop.engine = ins.engine
                        nop.sync_info = mybir.SyncInfo(on_wait=[w], on_update=[])
                        out.append(nop)
                    ins.sync_info = mybir.SyncInfo(
                        on_wait=[waits[-1]], on_update=list(si.on_update or [])
                    )
                out.append(ins)
            bb.instructions = out
    return cnt


def _build(use_f32r=True):
    import concourse.bass as bass
    import concourse.mybir as mybir
    import concourse.tile as tile

    f32 = mybir.dt.float32
    bf16 = mybir.dt.bfloat16
    f32r = mybir.dt.float32r
    Alu = mybir.AluOpType
    Act = mybir.ActivationFunctionType

    nc = bass.Bass("TRN2", target_bir_lowering=False, debug=False)

    mmdt = bf16

    imgw = nc.dram_tensor("imgw", [K * SLAB * WO], mmdt, kind="ExternalInput").ap()
    blob_d = nc.dram_tensor("blob", [128, BLOBC], mmdt, kind="ExternalInput").ap()
    lams_d = nc.dram_tensor("lams", [128, 2], f32, kind="ExternalInput").ap()
    out_d = nc.dram_tensor("out", [SLAB, Wimg], f32, kind="ExternalOutput").ap()
    obuf = nc.dram_tensor("obuf", [NCH * NPOS], bf16, kind="Internal").ap()

    def r(ap):
        return ap

    with tile.TileContext(nc) as tc:
        with contextlib.ExitStack() as ctx:
            wpool = ctx.enter_context(tc.tile_pool(name="w", bufs=1))
            big = ctx.enter_context(tc.tile_pool(name="big", bufs=1))
            pp = ctx.enter_context(tc.tile_pool(name="ps", bufs=2, space="PSUM"))
            ring = ctx.enter_context(tc.tile_pool(name="ring", bufs=4))
            stg = ctx.enter_context(tc.tile_pool(name="stg", bufs=2))

            blob = wpool.tile([128, BLOBC], mmdt)
            nc.sync.dma_start(blob[:], blob_d)
            o = {}
            col = 0
            for name, np_, nf in BLOB_SPEC:
                o[name] = (np_, col, nf)
                col += nf
            def bl(name, cast=None):
                np_, c0, nf = o[name]
                v = blob[0:np_, c0:c0 + nf]
                return v.bitcast(f32) if cast else v
            wac = bl("wac"); wacp = bl("wacp"); sbd = bl("sbd")
            id128 = bl("id128"); wwa = bl("wwa"); wwb = bl("wwb")
            j81 = bl("j81"); eshb = bl("eshb")
            lams = wpool.tile([128, 2], f32)
            nc.sync.dma_start(lams[:], lams_d)
            lam = lams[:, 0:1]
            nlam = lams[:, 1:2]

            icol = big.tile([NCH, NPOS], mmdt, tag="icol")
            c = big.tile([128, HALF], mmdt)
            gam = big.tile([128, HALF], mmdt)
            acc = big.tile([SLAB, Wimg], f32)
            ypool = ctx.enter_context(tc.tile_pool(name="y", bufs=4))

            # ---- im2col: host supplies img_w[kw] = slab[:, kw:kw+WO]; each
            # channel (kh, kw) = img_w[kw][kh:kh+ROWS] is one contiguous run.
            for hh in range(2):
                eng = nc.sync if hh == 0 else nc.scalar
                for q in range(2):
                    r0, r1 = hh * HALFR + q * (HALFR // 2), hh * HALFR + (q + 1) * (HALFR // 2)
                    eng.dma_start(
                        icol[:, r0 * WO:r1 * WO],
                        bass.AP(imgw.tensor, r0 * WO,
                                [[WO, K], [SLAB * WO, K], [1, (r1 - r0) * WO]]))

            # ---- encode: c = WAc @ I_col for both halves (B via col-tile 64)
            for si, sup in enumerate(SUPERS):
                ps = pp.tile([128, 2048], f32, tag="ps")
                c0s, c1s = sup[0][0], sup[-1][1]
                for jj, (c0, c1) in enumerate(sup):
                    n = c1 - c0
                    slb = ps[0:128, jj * CH: jj * CH + n]
                    nc.tensor.matmul(slb, r(wacp), r(icol[:, HALF + c0: HALF + c1]),
                                     start=True, stop=True)
                    sl = ps[0:F, jj * CH: jj * CH + n]
                    nc.tensor.matmul(sl, r(wac), r(icol[:, c0:c1]),
                                     start=True, stop=True)
                span = c1s - c0s
                nc.scalar.copy(c[:, c0s:c1s], ps[:, 0:span])
                # gamma0 = c - clip(c) directly after each superstep's c lands
                z = ypool.tile([128, 2048], mmdt, tag="z")
                nc.vector.tensor_scalar(z[:, 0:span], c[:, c0s:c1s],
                                        lam, nlam, Alu.min, Alu.max)
                nc.vector.tensor_tensor(gam[:, c0s:c1s], c[:, c0s:c1s],
                                        z[:, 0:span], Alu.subtract)

            # ---- 23 fused iterations: y = Id@c + S@gam; gam' = y - clip(y)
            for _t in range(UNF - 1):
                for sup in SUPERS:
                    ps = pp.tile([128, 2048], f32, tag="ps")
                    c0s, c1s = sup[0][0], sup[-1][1]
                    for jj, (c0, c1) in enumerate(sup):
                        nc.tensor.matmul(ps[:, jj * CH: jj * CH + (c1 - c0)],
                                         r(id128), r(c[:, c0:c1]),
                                         start=True, stop=False)
                    for jj, (c0, c1) in enumerate(sup):
                        nc.tensor.matmul(ps[:, jj * CH: jj * CH + (c1 - c0)],
                                         r(sbd), r(gam[:, c0:c1]),
                                         start=False, stop=True)
                    span = c1s - c0s
                    yt = ypool.tile([128, 2048], mmdt, tag="y")
                    nc.scalar.copy(yt[:, 0:span], ps[:, 0:span])
                    z = ypool.tile([128, 2048], mmdt, tag="z")
                    nc.vector.tensor_scalar(z[:, 0:span], yt[:, 0:span],
                                            lam, nlam, Alu.min, Alu.max)
                    nc.vector.tensor_tensor(gam[:, c0s:c1s], yt[:, 0:span],
                                            z[:, 0:span], Alu.subtract)

            # ---- decode: out_all = WW@a + WW@bn + (J/81)@I_col, stream to HBM
            ngrp = HALFR // DROWS  # 23 two-row chunks per half
            di = 0
            for half in range(2):
                ww = wwb if half else wwa
                for g0 in range(0, ngrp, 4):
                    ps = pp.tile([128, 2048], f32, tag="ps")
                    rg = ring.tile([NCH, 4 * DCH], bf16, tag="ring")
                    nch = 0
                    for jj, g in enumerate(range(g0, min(g0 + 4, ngrp))):
                        c0 = g * DCH
                        sl = ps[0:NCH, jj * CH: jj * CH + DCH]
                        nc.tensor.matmul(sl, r(ww),
                                         r(gam[:, c0:c0 + DCH]),
                                         start=True, stop=False)
                        nc.tensor.matmul(sl, r(j81),
                                         r(icol[:, half * HALF + c0: half * HALF + c0 + DCH]),
                                         start=False, stop=True)
                        if di % 2 == 0:
                            nc.scalar.copy(rg[:, jj * DCH:(jj + 1) * DCH], sl)
                        else:
                            nc.vector.tensor_copy(rg[:, jj * DCH:(jj + 1) * DCH], sl)
                        di += 1
                        nch += 1
                    r0 = half * HALFR + g0 * DROWS
                    dst = bass.AP(obuf.tensor, r0 * NCH * WO,
                                  [[WO, NCH], [NCH * WO, nch * DROWS], [1, WO]])
                    eng = nc.sync if (g0 // 4) % 2 == 0 else nc.scalar
                    eng.dma_start(dst, rg[:, 0:nch * DCH])

            # ---- col2im: one contiguous gather (reuses icol's SBUF slot),
            # kw-merge per kh in the free dim, then row-shift via 0/1
            # shift-matrix matmuls accumulating in PSUM.
            stall = big.tile([ROWS, NCH * WO], bf16, tag="icol")
            for kh in range(K):
                eng = (nc.sync, nc.scalar, nc.gpsimd)[kh % 3]
                eng.dma_start(
                    stall[:, kh * K * WO:(kh + 1) * K * WO],
                    bass.AP(obuf.tensor, kh * K * WO,
                            [[NCH * WO, ROWS], [1, K * WO]]))
            ops = pp.tile([128, 2048], f32, tag="ps")
            for kh in range(K):
                lhs = eshb[:, kh * SLAB:(kh + 1) * SLAB]
                for kw in range(K):
                    nc.tensor.matmul(
                        ops[0:SLAB, kw:kw + WO], lhs,
                        stall[:, (kh * K + kw) * WO:(kh * K + kw + 1) * WO],
                        start=(kh == 0 and kw == 0),
                        stop=(kh == K - 1 and kw == K - 1))
            nc.scalar.copy(acc[:], ops[0:SLAB, 0:Wimg])
            nc.sync.dma_start(out_d, acc[:])

    n = _split_multi_waits(nc, mybir)
    return nc


def _get_nc():
    if "nc" not in _STATE:
        _STATE["nc"] = _build(use_f32r=True)
    return _STATE["nc"]


def _make_in_maps(I, WA, WD, WW, lmbda):
    import ml_dtypes  # noqa: F401
    I = np.ascontiguousarray(np.asarray(I, np.float32))
    WA = np.asarray(WA, np.float32)
    WD = np.asarray(WD, np.float32)
    WW = np.asarray(WW, np.float32)
    lam = np.asarray(lmbda, np.float32).reshape(F)
    assert I.shape == (B, 1, H, Wimg)

    WAc = (WA - WA.mean(axis=1, keepdims=True)).astype(np.float32)  # [64,81]
    S = (np.eye(F, dtype=np.float32) - WA @ WD).astype(np.float32)  # [64,64]
    sbd = np.zeros((128, 128), np.float32)
    sbd[0:F, 0:F] = S.T
    sbd[F:128, F:128] = S.T
    id128 = np.eye(128, dtype=np.float32)
    wacp = np.zeros((81, 128), np.float32)
    wacp[:, F:128] = WAc.T
    wwa = np.zeros((128, 81), np.float32)
    wwa[0:F, :] = WW.T
    wwb = np.zeros((128, 81), np.float32)
    wwb[F:128, :] = WW.T
    j81 = np.full((NCH, NCH), 1.0 / NCH, np.float32)
    lam128 = np.concatenate([lam, lam]).reshape(128, 1).astype(np.float32)
    esh = np.zeros((ROWS, K * SLAB), np.float32)  # lhsT per kh: E[r, y]=1 iff y=r+kh
    for kh in range(K):
        for rr in range(ROWS):
            esh[rr, kh * SLAB + rr + kh] = 1.0
    vals = {"wac": WAc.T, "wacp": wacp, "sbd": sbd, "id128": id128,
            "wwa": wwa, "wwb": wwb, "j81": j81, "eshb": esh}
    blob = np.zeros((128, BLOBC), np.float32)
    col = 0
    for name, np_, nf in BLOB_SPEC:
        v = np.asarray(vals[name], np.float32)
        assert v.shape == (np_, nf), (name, v.shape)
        blob[0:np_, col:col + nf] = v
        col += nf
    lams = np.concatenate([lam128, -lam128], axis=1).astype(np.float32)

    shared = {"blob": blob.astype(ml_dtypes.bfloat16), "lams": lams}
    in_maps = []
    for core in range(N_CORES):
        b, h = core // 2, core % 2
        r0 = h * ROWS
        slab = I[b, 0, r0:r0 + SLAB, :]
        imgw = np.stack([slab[:, kw:kw + WO] for kw in range(K)], axis=0)
        in_maps.append({"imgw": np.ascontiguousarray(imgw).reshape(-1).astype(
            ml_dtypes.bfloat16), **shared})
    return in_maps


def _unshard(results):
    cnt = np.zeros((H, Wimg), np.float32)
    for kh in range(K):
        for kw in range(K):
            cnt[kh:kh + HO, kw:kw + WO] += 1.0
    out = np.zeros((B, 1, H, Wimg), np.float32)
    for b in range(B):
        acc = np.zeros((H, Wimg), np.float32)
        acc[0:SLAB, :] += results[2 * b]["out"]
        acc[ROWS:ROWS + SLAB, :] += results[2 * b + 1]["out"]
        out[b, 0] = acc / cnt
    return out


def kernel(I, WA, WD, WW, lmbda, kernel_size=9, stride=1, unfoldings=24, **_kw):
    from concourse import bass_utils

    assert int(kernel_size) == K and int(stride) == 1 and int(unfoldings) == UNF
    in_maps = _make_in_maps(I, WA, WD, WW, lmbda)
    nc = _get_nc()
    last = None
    for _attempt in range(3):
        try:
            res = bass_utils.run_bass_kernel_spmd(
                nc, in_maps, core_ids=list(range(N_CORES)))
            return _unshard(res.results)
        except Exception as e:  # transient NRT device errors: retry
            last = e
    raise last


# revision 22
# speedup vs baseline: 1.1261x; 1.1261x over previous
"""LISTA (learned ISTA) sparse-coding forward pass on 8 Trainium2 NeuronCores.

Problem: I [4,1,192,192] -> im2col(9x9) -> 24 soft-thresholded iterations over
64 filters -> decode -> col2im overlap-add average -> [4,1,192,192].

Sharding: 8 cores = 4 images x 2 position-row halves (92 rows of 184 positions
each). Each core computes its full LISTA pipeline plus the col2im partial sums
for its 100-row output slab; the host merges the 8-row seams between the two
slabs of each image and divides by the overlap counts (pure unshard glue).

Algebra used (exact rewrites of the reference up to fp assoc.):
  - mean-subtraction folded into encoder:  c = WAc @ I_col,
      WAc = WA - rowmean(WA)  (since mean_patch = (1/81) * ones^T I_col)
  - iteration fused:  gamma_{t+1} = soft(S @ gamma_t + c),  S = I - WA@WD
  - gamma kept as a + bneg with a = relu(y-l) >= 0, bneg = min(y+l, 0) <= 0
    (soft(y) = a + bneg), so the subtraction never needs its own pass:
      y_{t+1} = Id@c + S@a_t + S@bneg_t   (3 accumulating PE passes)
  - decode: out_all = WW@a + WW@bneg + (J/81) @ I_col   (mean add-back)
"""

import contextlib
import numpy as np

# ---------------------------------------------------------------- constants
B, H, Wimg = 4, 192, 192
K = 9
F = 64
NCH = K * K  # 81
HO = H - K + 1  # 184
WO = Wimg - K + 1  # 184
UNF = 24
N_CORES = 8

ROWS = HO // 2  # 92 position rows per core
SLAB = ROWS + K - 1  # 100 image/output rows per core
NPOS = ROWS * WO  # 16928 positions per core
HALFR = ROWS // 2  # 46 rows per block-diag half
HALF = HALFR * WO  # 8464 columns per half

CH = 512
CHUNKS = [(i * CH, min((i + 1) * CH, HALF)) for i in range((HALF + CH - 1) // CH)]
SUPERS = [CHUNKS[i : i + 2] for i in range(0, len(CHUNKS), 2)]
DROWS = 2  # decode chunk = 2 position rows
DCH = DROWS * WO  # 368 columns

# weight blob layout: (name, partitions, cols) — bf16
BLOB_SPEC = [
    ("wac", NCH, F), ("wacp", NCH, 128), ("sbd", 128, 128), ("id128", 128, 128),
    ("wwa", 128, NCH), ("wwb", 128, NCH), ("j81", NCH, NCH),
    ("eshb", ROWS, K * SLAB),
]
BLOBC = sum(nf for _, _, nf in BLOB_SPEC)

_STATE = {}


def _split_multi_waits(nc, mybir):
    """This walrus build supports a single sync-wait slot per instruction.
    Move extra waits onto preceding same-engine no-ops (same semantics:
    program order on one engine; all waits clear before the instruction)."""
    cnt = 0
    for fn in nc.m.functions:
        for bb in fn.blocks:
            insts = bb.instructions
            need = False
            for ins in insts:
                si = ins.sync_info
                if si is not None and si.on_wait is not None and len(si.on_wait) > 1:
                    need = True
                    break
            if not need:
                continue
            out = []
            for ins in insts:
                si = ins.sync_info
                if si is not None and si.on_wait is not None and len(si.on_wait) > 1:
                    waits = list(si.on_wait)
                    for w in waits[:-1]:
                        cnt += 1
                        nop = mybir.InstNoOp(name=f"wsplit-{cnt}", ins=[], outs=[])
                        nop.engine = ins.engine
                        nop.sync_info = mybir.SyncInfo(on_wait=[w], on_update=[])
                        out.append(nop)
                    ins.sync_info = mybir.SyncInfo(
                        on_wait=[waits[-1]], on_update=list(si.on_update or [])
                    )
                out.append(ins)
            bb.instructions = out
    return cnt


def _build(use_f32r=True):
    import concourse.bass as bass
    import concourse.mybir as mybir
    import concourse.tile as tile

    f32 = mybir.dt.float32
    bf16 = mybir.dt.bfloat16
    f32r = mybir.dt.float32r
    Alu = mybir.AluOpType
    Act = mybir.ActivationFunctionType

    nc = bass.Bass("TRN2", target_bir_lowering=False, debug=False)

    mmdt = bf16

    imgw = nc.dram_tensor("imgw", [K * SLAB * WO], mmdt, kind="ExternalInput").ap()
    blob_d = nc.dram_tensor("blob", [128, BLOBC], mmdt, kind="ExternalInput").ap()
    lams_d = nc.dram_tensor("lams", [128, 2], f32, kind="ExternalInput").ap()
    out_d = nc.dram_tensor("out", [SLAB, Wimg], f32, kind="ExternalOutput").ap()
    obuf = nc.dram_tensor("obuf", [NCH * NPOS], bf16, kind="Internal").ap()

    def r(ap):
        return ap

    with tile.TileContext(nc) as tc:
        with contextlib.ExitStack() as ctx:
            wpool = ctx.enter_context(tc.tile_pool(name="w", bufs=1))
            big = ctx.enter_context(tc.tile_pool(name="big", bufs=1))
            pp = ctx.enter_context(tc.tile_pool(name="ps", bufs=4, space="PSUM"))
            ring = ctx.enter_context(tc.tile_pool(name="ring", bufs=4))
            stg = ctx.enter_context(tc.tile_pool(name="stg", bufs=2))

            blob = wpool.tile([128, BLOBC], mmdt)
            nc.sync.dma_start(blob[:], blob_d)
            o = {}
            col = 0
            for name, np_, nf in BLOB_SPEC:
                o[name] = (np_, col, nf)
                col += nf
            def bl(name, cast=None):
                np_, c0, nf = o[name]
                v = blob[0:np_, c0:c0 + nf]
                return v.bitcast(f32) if cast else v
            wac = bl("wac"); wacp = bl("wacp"); sbd = bl("sbd")
            id128 = bl("id128"); wwa = bl("wwa"); wwb = bl("wwb")
            j81 = bl("j81"); eshb = bl("eshb")
            lams = wpool.tile([128, 2], f32)
            nc.sync.dma_start(lams[:], lams_d)
            lam = lams[:, 0:1]
            nlam = lams[:, 1:2]

            icol = big.tile([NCH, NPOS], mmdt, tag="icol")
            c = big.tile([128, HALF], mmdt)
            gam = big.tile([128, HALF], mmdt)
            acc = big.tile([SLAB, Wimg], f32)
            ypool = ctx.enter_context(tc.tile_pool(name="y", bufs=4))

            # ---- im2col: host supplies img_w[kw] = slab[:, kw:kw+WO]; each
            # channel (kh, kw) = img_w[kw][kh:kh+ROWS] is one contiguous run.
            for hh in range(2):
                eng = nc.sync if hh == 0 else nc.scalar
                for q in range(2):
                    r0, r1 = hh * HALFR + q * (HALFR // 2), hh * HALFR + (q + 1) * (HALFR // 2)
                    eng.dma_start(
                        icol[:, r0 * WO:r1 * WO],
                        bass.AP(imgw.tensor, r0 * WO,
                                [[WO, K], [SLAB * WO, K], [1, (r1 - r0) * WO]]))

            # ---- encode: c = WAc @ I_col for both halves (B via col-tile 64)
            for si, sup in enumerate(SUPERS):
                ps = pp.tile([128, 1024], f32, tag="ps")
                c0s, c1s = sup[0][0], sup[-1][1]
                for jj, (c0, c1) in enumerate(sup):
                    n = c1 - c0
                    slb = ps[0:128, jj * CH: jj * CH + n]
                    nc.tensor.matmul(slb, r(wacp), r(icol[:, HALF + c0: HALF + c1]),
                                     start=True, stop=True)
                    sl = ps[0:F, jj * CH: jj * CH + n]
                    nc.tensor.matmul(sl, r(wac), r(icol[:, c0:c1]),
                                     start=True, stop=True)
                span = c1s - c0s
                nc.scalar.copy(c[:, c0s:c1s], ps[:, 0:span])
                # gamma0 = c - clip(c) directly after each superstep's c lands
                z = ypool.tile([128, 1024], mmdt, tag="z0")
                nc.vector.tensor_scalar(z[:, 0:span], c[:, c0s:c1s],
                                        lam, nlam, Alu.min, Alu.max)
                nc.vector.tensor_tensor(gam[:, c0s:c1s], c[:, c0s:c1s],
                                        z[:, 0:span], Alu.subtract)

            # ---- 23 fused iterations: y = Id@c + S@gam; gam' = y - clip(y)
            # ACT copies y out per 1024-superstep; DVE clip+sub batched in
            # 2048 pairs to amortize per-op overheads.
            for _t in range(UNF - 1):
                for sp in range(0, len(SUPERS), 2):
                    pair = SUPERS[sp:sp + 2]
                    p0 = pair[0][0][0]
                    yt = ypool.tile([128, 2048], mmdt, tag="y")
                    off = 0
                    for sup in pair:
                        ps = pp.tile([128, 1024], f32, tag="ps")
                        c0s, c1s = sup[0][0], sup[-1][1]
                        for jj, (c0, c1) in enumerate(sup):
                            nc.tensor.matmul(ps[:, jj * CH: jj * CH + (c1 - c0)],
                                             r(id128), r(c[:, c0:c1]),
                                             start=True, stop=False)
                        for jj, (c0, c1) in enumerate(sup):
                            nc.tensor.matmul(ps[:, jj * CH: jj * CH + (c1 - c0)],
                                             r(sbd), r(gam[:, c0:c1]),
                                             start=False, stop=True)
                        span = c1s - c0s
                        nc.scalar.copy(yt[:, off:off + span], ps[:, 0:span])
                        off += span
                    z = ypool.tile([128, 2048], mmdt, tag="z")
                    nc.vector.tensor_scalar(z[:, 0:off], yt[:, 0:off],
                                            lam, nlam, Alu.min, Alu.max)
                    nc.vector.tensor_tensor(gam[:, p0:p0 + off], yt[:, 0:off],
                                            z[:, 0:off], Alu.subtract)

            # ---- decode: out_all = WW@a + WW@bn + (J/81)@I_col, stream to HBM
            ngrp = HALFR // DROWS  # 23 two-row chunks per half
            di = 0
            for half in range(2):
                ww = wwb if half else wwa
                for g0 in range(0, ngrp, 2):
                    ps = pp.tile([128, 1024], f32, tag="ps")
                    rg = ring.tile([NCH, 2 * DCH], bf16, tag="ring")
                    nch = 0
                    for jj, g in enumerate(range(g0, min(g0 + 2, ngrp))):
                        c0 = g * DCH
                        sl = ps[0:NCH, jj * CH: jj * CH + DCH]
                        nc.tensor.matmul(sl, r(ww),
                                         r(gam[:, c0:c0 + DCH]),
                                         start=True, stop=False)
                        nc.tensor.matmul(sl, r(j81),
                                         r(icol[:, half * HALF + c0: half * HALF + c0 + DCH]),
                                         start=False, stop=True)
                        if di % 2 == 0:
                            nc.scalar.copy(rg[:, jj * DCH:(jj + 1) * DCH], sl)
                        else:
                            nc.vector.tensor_copy(rg[:, jj * DCH:(jj + 1) * DCH], sl)
                        di += 1
                        nch += 1
                    r0 = half * HALFR + g0 * DROWS
                    dst = bass.AP(obuf.tensor, r0 * NCH * WO,
                                  [[WO, NCH], [NCH * WO, nch * DROWS], [1, WO]])
                    eng = nc.sync if (g0 // 2) % 2 == 0 else nc.scalar
                    eng.dma_start(dst, rg[:, 0:nch * DCH])

            # ---- col2im: one contiguous gather (reuses icol's SBUF slot),
            # kw-merge per kh in the free dim, then row-shift via 0/1
            # shift-matrix matmuls accumulating in PSUM.
            stall = big.tile([ROWS, NCH * WO], bf16, tag="icol")
            for kh in range(K):
                eng = (nc.sync, nc.scalar, nc.gpsimd)[kh % 3]
                eng.dma_start(
                    stall[:, kh * K * WO:(kh + 1) * K * WO],
                    bass.AP(obuf.tensor, kh * K * WO,
                            [[NCH * WO, ROWS], [1, K * WO]]))
            ops = pp.tile([128, 1024], f32, tag="ps")
            for kh in range(K):
                lhs = eshb[:, kh * SLAB:(kh + 1) * SLAB]
                for kw in range(K):
                    nc.tensor.matmul(
                        ops[0:SLAB, kw:kw + WO], lhs,
                        stall[:, (kh * K + kw) * WO:(kh * K + kw + 1) * WO],
                        start=(kh == 0 and kw == 0),
                        stop=(kh == K - 1 and kw == K - 1))
            nc.scalar.copy(acc[:], ops[0:SLAB, 0:Wimg])
            nc.sync.dma_start(out_d, acc[:])

    n = _split_multi_waits(nc, mybir)
    return nc


def _get_nc():
    if "nc" not in _STATE:
        _STATE["nc"] = _build(use_f32r=True)
    return _STATE["nc"]


def _make_in_maps(I, WA, WD, WW, lmbda):
    import ml_dtypes  # noqa: F401
    I = np.ascontiguousarray(np.asarray(I, np.float32))
    WA = np.asarray(WA, np.float32)
    WD = np.asarray(WD, np.float32)
    WW = np.asarray(WW, np.float32)
    lam = np.asarray(lmbda, np.float32).reshape(F)
    assert I.shape == (B, 1, H, Wimg)

    WAc = (WA - WA.mean(axis=1, keepdims=True)).astype(np.float32)  # [64,81]
    S = (np.eye(F, dtype=np.float32) - WA @ WD).astype(np.float32)  # [64,64]
    sbd = np.zeros((128, 128), np.float32)
    sbd[0:F, 0:F] = S.T
    sbd[F:128, F:128] = S.T
    id128 = np.eye(128, dtype=np.float32)
    wacp = np.zeros((81, 128), np.float32)
    wacp[:, F:128] = WAc.T
    wwa = np.zeros((128, 81), np.float32)
    wwa[0:F, :] = WW.T
    wwb = np.zeros((128, 81), np.float32)
    wwb[F:128, :] = WW.T
    j81 = np.full((NCH, NCH), 1.0 / NCH, np.float32)
    lam128 = np.concatenate([lam, lam]).reshape(128, 1).astype(np.float32)
    esh = np.zeros((ROWS, K * SLAB), np.float32)  # lhsT per kh: E[r, y]=1 iff y=r+kh
    for kh in range(K):
        for rr in range(ROWS):
            esh[rr, kh * SLAB + rr + kh] = 1.0
    vals = {"wac": WAc.T, "wacp": wacp, "sbd": sbd, "id128": id128,
            "wwa": wwa, "wwb": wwb, "j81": j81, "eshb": esh}
    blob = np.zeros((128, BLOBC), np.float32)
    col = 0
    for name, np_, nf in BLOB_SPEC:
        v = np.asarray(vals[name], np.float32)
        assert v.shape == (np_, nf), (name, v.shape)
        blob[0:np_, col:col + nf] = v
        col += nf
    lams = np.concatenate([lam128, -lam128], axis=1).astype(np.float32)

    shared = {"blob": blob.astype(ml_dtypes.bfloat16), "lams": lams}
    in_maps = []
    for core in range(N_CORES):
        b, h = core // 2, core % 2
        r0 = h * ROWS
        slab = I[b, 0, r0:r0 + SLAB, :]
        imgw = np.stack([slab[:, kw:kw + WO] for kw in range(K)], axis=0)
        in_maps.append({"imgw": np.ascontiguousarray(imgw).reshape(-1).astype(
            ml_dtypes.bfloat16), **shared})
    return in_maps


def _unshard(results):
    cnt = np.zeros((H, Wimg), np.float32)
    for kh in range(K):
        for kw in range(K):
            cnt[kh:kh + HO, kw:kw + WO] += 1.0
    out = np.zeros((B, 1, H, Wimg), np.float32)
    for b in range(B):
        acc = np.zeros((H, Wimg), np.float32)
        acc[0:SLAB, :] += results[2 * b]["out"]
        acc[ROWS:ROWS + SLAB, :] += results[2 * b + 1]["out"]
        out[b, 0] = acc / cnt
    return out


def kernel(I, WA, WD, WW, lmbda, kernel_size=9, stride=1, unfoldings=24, **_kw):
    from concourse import bass_utils

    assert int(kernel_size) == K and int(stride) == 1 and int(unfoldings) == UNF
    in_maps = _make_in_maps(I, WA, WD, WW, lmbda)
    nc = _get_nc()
    last = None
    for _attempt in range(3):
        try:
            res = bass_utils.run_bass_kernel_spmd(
                nc, in_maps, core_ids=list(range(N_CORES)))
            return _unshard(res.results)
        except Exception as e:  # transient NRT device errors: retry
            last = e
    raise last
